# revision 14
# baseline (speedup 1.0000x reference)
"""Trainium2 Bass kernel for Conf-MPU loss (nn_Conf_MPULoss).

Strategy: the loss is a streaming reduction over N rows x 5 classes down to a
handful of per-class accumulators, followed by a trivial scalar combination.

Host side:
  - rows are partitioned by label t into 5 class groups (stable), split evenly
    across 8 cores, each per-core class segment padded to S = 128*R rows.
  - pivot transform: for a segment with label c the host sends the 4 columns
    y_i = x_i - x_c (i != c) as bf16, class-major blocks of width R. This
    both (a) drops DMA from 20B/row fp32 to 8B/row bf16 and (b) makes
    Zc1 := 1 + sum_i e^{y_i} = Z / e^{x_c} = 1/p_c, so the per-row reciprocal
    1/p_c needed by the conf term is free (no divide, no second exp).
  - the C-length accumulators from all cores are reduced on host (fp64) and
    combined into the final scalar.

Device side (per core, SPMD over 8 cores), per class segment [128, 4R] bf16
(blocks b=0..3 of width R; for c<4 block 3 is z4 = x4 - xc):
    E    = exp(Y)                      ScalarE (one LUT set: exp+ln)
    T1   = E[:,0:2R] + E[:,2R:4R]      DVE TT bf16 (2x mode, dense)
    Zc1  = (T1a + 1) + T1b             DVE STT bf16
    LnZ  = ln(Zc1)                     ScalarE, accum-> sum(ln) [c<4]
    c < 4 (LnZ = dt = -log p_c):
      D4 = LnZ - z4 (= -log p_neg)     DVE STT, accum-> sum(D4)
      U  = Zc1 * D4                    GpSimd TT
      num: (Zc1 < 2) * U, accum        DVE STT   [p_c > 0.5 <=> Zc1 < 2]
      den: (Zc1 < 2), accum            DVE tensor_scalar (4x mode)
      (sd_c = sum(z4) = sum(ln) - sum(D4), pads removed exactly on host)
    c == 4, two half-tiles (LnZ = d4 = -log p_neg):
      T2 = max(Ea, Eb); EM = max(..)   GpSimd TT
      W4 = 2*EM - Zc1                  DVE STT   [all pos p<=0.5 <=> W4<=0]
      M  = (Zc1 >= 2) * LnZ            DVE STT   [p_neg<=0.5 <=> Zc1>=2]
      li: (W4 <= 0) * M, accum         DVE STT

Sentinel pads: +20 in all 4 cols for c<4 (mask off, z4=20 exactly corrected
on host), -20 for the negative segment (Zc1=1 -> everything 0).
"""

import numpy as np
import ml_dtypes

import concourse.bacc as bacc
import concourse.mybir as mybir
import concourse.tile as tile
from concourse import bass_utils

F32 = mybir.dt.float32
BF16 = mybir.dt.bfloat16
Alu = mybir.AluOpType
Act = mybir.ActivationFunctionType

P = 128
NCLS = 5
N_CORES = 8
NCOLS = 22  # 4 pos classes * [numA, numB, den, sumD4, sumLn] + [liA, liB]

BF = ml_dtypes.bfloat16

_PROGRAM_CACHE: dict[int, object] = {}


def _restrict_act_tables(arch: str):
    """Confine Exp/Ln to the natural_log_exp_and_others set so the act-table
    pass emits a single ACT_TABLE_LOAD instead of thrashing between sets."""
    from concourse import hw_specs

    tables = hw_specs.get_activation_tables(arch)
    if "natural_log_exp_and_others" not in tables:
        return
    for name, funcs in tables.items():
        if name != "natural_log_exp_and_others":
            funcs.discard(Act.Exp)
            funcs.discard(Act.Ln)


def _build_program(R: int):
    """Build + compile the per-core Bass program for segment width 4R."""
    assert R % 2 == 0
    H = R // 2  # half-tile block width for the negative segment
    nc = bacc.Bacc("TRN2", debug=False, num_devices=N_CORES)
    _restrict_act_tables(nc.m.arch)
    y_d = nc.dram_tensor("y", [NCLS, P, 4 * R], BF16, kind="ExternalInput").ap()
    st_d = nc.dram_tensor("stats", [P, NCOLS], F32, kind="ExternalOutput").ap()

    with tile.TileContext(nc) as tc:
        with (
            tc.tile_pool(name="io", bufs=6) as iop,
            tc.tile_pool(name="ep", bufs=3) as epp,
            tc.tile_pool(name="wk", bufs=3) as wp,
            tc.tile_pool(name="st", bufs=1) as sp,
        ):
            stats = sp.tile([P, NCOLS], F32)
            nc.vector.memset(stats, 0.0)
            ONES = sp.tile([P, R], BF16)
            nc.vector.memset(ONES, 1.0)

            def neg_half(h, split_dma):
                """One half-tile of the negative segment (label = class 4)."""
                W = 2 * R * h  # column offset of this half in y_d[4]
                Y = iop.tile([P, 2 * R], BF16, tag=f"y4{h}", name="Y")
                E = epp.tile([P, 2 * R], BF16, tag="e4", name="E")
                if split_dma:
                    # two DMA+exp chunks so ScalarE starts ~1.5us earlier
                    for lo, hi in ((0, R), (R, 2 * R)):
                        nc.sync.dma_start(
                            out=Y[:, lo:hi], in_=y_d[4][:, W + lo : W + hi]
                        )
                        nc.scalar.activation(E[:, lo:hi], Y[:, lo:hi], Act.Exp)
                else:
                    nc.sync.dma_start(out=Y, in_=y_d[4][:, W : W + 2 * R])
                    nc.scalar.activation(E, Y, Act.Exp)
                T1 = wp.tile([P, R], BF16, tag="t1h", name="T1")
                nc.vector.tensor_tensor(
                    out=T1, in0=E[:, 0:R], in1=E[:, R : 2 * R], op=Alu.add
                )
                Z = wp.tile([P, H], BF16, tag="zh", name="Z")
                nc.vector.tensor_tensor(
                    out=Z, in0=T1[:, 0:H], in1=T1[:, H:R], op=Alu.add
                )
                # D4 = -log p_neg = ln(Z' + 1)
                D4 = wp.tile([P, H], BF16, tag="lnzh", name="D4")
                nc.scalar.activation(D4, Z, Act.Ln, bias=1.0)
                T2 = wp.tile([P, R], BF16, tag="t2", name="T2")
                nc.vector.tensor_tensor(
                    out=T2, in0=E[:, 0:R], in1=E[:, R : 2 * R], op=Alu.max
                )
                EM = wp.tile([P, H], BF16, tag="em", name="EM")
                nc.vector.tensor_tensor(
                    out=EM, in0=T2[:, 0:H], in1=T2[:, H:R], op=Alu.max
                )
                # W4 = 2*EM - Z'; all positive p <= 0.5  <=>  W4 <= 1
                W4 = wp.tile([P, H], BF16, tag="w4", name="W4")
                nc.vector.scalar_tensor_tensor(
                    out=W4, in0=EM, scalar=2.0, in1=Z,
                    op0=Alu.mult, op1=Alu.subtract,
                )
                # M = (p_neg <= 0.5) * D4  <=>  (Z' >= 1) * D4
                M = wp.tile([P, H], BF16, tag="m", name="M")
                nc.vector.scalar_tensor_tensor(
                    out=M, in0=Z, scalar=1.0, in1=D4,
                    op0=Alu.is_ge, op1=Alu.mult,
                )
                G = wp.tile([P, H], BF16, tag="gh", name="G")
                nc.vector.scalar_tensor_tensor(
                    out=G, in0=W4, scalar=1.0, in1=M,
                    op0=Alu.is_le, op1=Alu.mult,
                    accum_out=stats[:, 20 + h : 21 + h],
                )

            def pos_seg(c):
                """One positive-class segment (label = class c < 4)."""
                Y = iop.tile([P, 4 * R], BF16, tag="y", name="Y")
                nc.sync.dma_start(out=Y, in_=y_d[c])
                E = epp.tile([P, 4 * R], BF16, tag="e", name="E")
                nc.scalar.activation(E, Y, Act.Exp)
                T1 = wp.tile([P, 2 * R], BF16, tag="t1", name="T1")
                nc.vector.tensor_tensor(
                    out=T1, in0=E[:, 0 : 2 * R], in1=E[:, 2 * R : 4 * R], op=Alu.add
                )
                Z = wp.tile([P, R], BF16, tag="z", name="Z")
                nc.vector.tensor_tensor(
                    out=Z, in0=T1[:, 0:R], in1=T1[:, R : 2 * R], op=Alu.add
                )
                # dt = -log p_c = ln(Z' + 1); accum -> sum(dt)
                DT = wp.tile([P, R], BF16, tag="lnz", name="DT")
                nc.scalar.activation(
                    DT, Z, Act.Ln, bias=1.0,
                    accum_out=stats[:, 5 * c + 4 : 5 * c + 5],
                )
                # D4 = -log p_neg = dt - z4; accum -> sum(dt - z4)
                D4 = wp.tile([P, R], BF16, tag="d4", name="D4")
                nc.vector.scalar_tensor_tensor(
                    out=D4,
                    in0=DT,
                    scalar=1.0,
                    in1=Y[:, 3 * R : 4 * R],
                    op0=Alu.mult,
                    op1=Alu.subtract,
                    accum_out=stats[:, 5 * c + 3 : 5 * c + 4],
                )
                # num = sum(mask * (Z'+1) * D4) split as
                #   numA = sum(mask * Z'*D4) + numB = sum(mask * D4)
                UB = wp.tile([P, R], BF16, tag="u", name="UB")
                nc.gpsimd.tensor_tensor(out=UB, in0=Z, in1=D4, op=Alu.mult)
                # p_c > 0.5  <=>  Z' < 1
                G = wp.tile([P, R], BF16, tag="g", name="G")
                nc.vector.scalar_tensor_tensor(
                    out=G,
                    in0=Z,
                    scalar=1.0,
                    in1=UB,
                    op0=Alu.is_lt,
                    op1=Alu.mult,
                    accum_out=stats[:, 5 * c : 5 * c + 1],
                )
                Gb = wp.tile([P, R], BF16, tag="gb", name="Gb")
                nc.vector.scalar_tensor_tensor(
                    out=Gb,
                    in0=Z,
                    scalar=1.0,
                    in1=D4,
                    op0=Alu.is_lt,
                    op1=Alu.mult,
                    accum_out=stats[:, 5 * c + 1 : 5 * c + 2],
                )
                if c < 2:
                    # den on DVE
                    Gd = wp.tile([P, R], BF16, tag="gd", name="Gd")
                    nc.vector.scalar_tensor_tensor(
                        out=Gd,
                        in0=Z,
                        scalar=1.0,
                        in1=ONES,
                        op0=Alu.is_lt,
                        op1=Alu.mult,
                        accum_out=stats[:, 5 * c + 2 : 5 * c + 3],
                    )
                else:
                    # den on ScalarE: sum(sign(1 - Z')) -> den = (acc + S)/2
                    Gs = wp.tile([P, R], BF16, tag="gs", name="Gs")
                    nc.scalar.activation(
                        Gs, Z, Act.Sign, scale=-1.0, bias=1.0,
                        accum_out=stats[:, 5 * c + 2 : 5 * c + 3],
                    )

            # one short half first (primes the pipeline), one short half last
            # (short tail: its chain has no GpSimd hop)
            neg_half(0, split_dma=True)
            for c in range(4):
                pos_seg(c)
            neg_half(1, split_dma=False)
            nc.sync.dma_start(out=st_d, in_=stats)
    nc.compile()
    return nc


def _get_program(R: int):
    if R not in _PROGRAM_CACHE:
        _PROGRAM_CACHE[R] = _build_program(R)
    return _PROGRAM_CACHE[R]


def _prepare_inputs(x: np.ndarray, t: np.ndarray):
    """Sort rows by class, pivot-transform, shard across cores, pad segments.
    Returns (in_maps, counts, n_pad, R)."""
    N = x.shape[0]
    t64 = t.astype(np.int64, copy=False)
    counts = np.bincount(t64, minlength=NCLS).astype(np.int64)

    # per-core per-class row counts (even split of each class across cores)
    n_ck = np.zeros((NCLS, N_CORES), dtype=np.int64)
    for c in range(NCLS):
        q, r = divmod(int(counts[c]), N_CORES)
        n_ck[c] = q
        n_ck[c, :r] += 1

    R = int(max(8, -(-int(n_ck.max()) // P)))
    R = (R + 1) // 2 * 2  # even: negative segment splits into two halves
    S = P * R

    order = np.argsort(t64, kind="stable")
    xs = np.ascontiguousarray(x[order], dtype=np.float32)
    starts = np.concatenate([[0], np.cumsum(counts)])

    ycores = np.empty((N_CORES, NCLS, P, 4 * R), dtype=BF)
    for c in range(NCLS):
        off = int(starts[c])
        cols = [i for i in range(NCLS) if i != c]
        if c < 4:
            cols = [i for i in cols if i != 4] + [4]  # z4 in the last block
        pad = np.float32(20.0 if c < 4 else -20.0)
        for k in range(N_CORES):
            n = int(n_ck[c, k])
            ys = np.full((S, 4), pad, dtype=np.float32)
            if n:
                seg = xs[off : off + n]
                ys[:n] = seg[:, cols] - seg[:, c : c + 1]
                off += n
            if c < 4:
                # [S, 4] -> [128, 4R] class-major blocks of width R
                ycores[k, c] = (
                    ys.reshape(P, R, 4).transpose(0, 2, 1).reshape(P, 4 * R)
                )
            else:
                # two half-tiles, each [128, 4H] with H = R//2
                H = R // 2
                for h in range(2):
                    half = ys[h * S // 2 : (h + 1) * S // 2]
                    ycores[k, c, :, h * 2 * R : (h + 1) * 2 * R] = (
                        half.reshape(P, H, 4).transpose(0, 2, 1).reshape(P, 4 * H)
                    )

    in_maps = [{"y": ycores[k]} for k in range(N_CORES)]
    n_pad = N_CORES * S - counts  # per class, summed over cores
    return in_maps, counts, n_pad, R


def _combine(stats_list, counts, n_pad, N, R):
    """Host all-reduce of the C-length accumulators + final scalar combination."""
    st = np.zeros(NCOLS, dtype=np.float64)
    for s in stats_list:
        st += s.astype(np.float64).sum(axis=0)

    counts = counts.astype(np.float64)
    # exact per-pad contribution to sum_ln - sum_d4, replicating device math:
    # pad rows are +20 in all 4 cols; the Ln accum taps pre-bf16-round fp32
    # while D4 subtracts the bf16-rounded dt, so the pad residual is
    # dt_f32 - bf16(dt_f32) + 20 (z4_pad = 20 is bf16-exact).
    e = np.float32(np.exp(np.float32(20.0))).astype(BF).astype(np.float32)
    t1 = (e + e).astype(BF).astype(np.float32)
    zp = (t1 + t1).astype(BF).astype(np.float32)
    dtf = np.float64(np.log1p(np.float64(zp)))
    dtb = np.float64(np.float32(dtf).astype(BF).astype(np.float64))
    pad_res = (dtf - dtb) + 20.0

    r13 = 0.0  # risk1 - risk3
    r2 = 0.0
    S_total = float(N_CORES * P * R)  # rows per class segment across cores
    for c in range(4):
        num = st[5 * c + 0] + st[5 * c + 1]  # numA + numB
        den = st[5 * c + 2]
        if c >= 2:
            den = (den + S_total) / 2.0  # sign-sum -> count
        sum_d4 = st[5 * c + 3]
        sum_ln = st[5 * c + 4]
        sd = (sum_ln - sum_d4) - pad_res * float(n_pad[c])  # sum_{t=c}(x4 - xc)
        prior = counts[c] / N
        r13 += prior * sd / max(1.0, counts[c])
        r2 += prior * num / max(den, 1.0)
    li = st[20] + st[21]
    r4 = li / max(1.0, counts[4])

    pos = 4.0 * (r13 + r2)
    if pos < 0.0:
        pos = 0.0
    return np.float32(pos + r4)


def run_device(in_maps, R, trace=False, **kw):
    nc = _get_program(R)
    res = bass_utils.run_bass_kernel_spmd(
        nc, in_maps, core_ids=list(range(N_CORES)), trace=trace, **kw
    )
    return res


def kernel(x: np.ndarray, t: np.ndarray) -> np.ndarray:
    x = np.asarray(x, dtype=np.float32)
    t = np.asarray(t)
    N = x.shape[0]
    in_maps, counts, n_pad, R = _prepare_inputs(x, t)
    res = run_device(in_maps, R)
    stats_list = [res.results[k]["stats"] for k in range(N_CORES)]
    return _combine(stats_list, counts, n_pad, N, R)


# revision 17
# speedup vs baseline: 1.1223x; 1.1223x over previous
"""Trainium2 Bass kernel for Conf-MPU loss (nn_Conf_MPULoss).

Strategy: the loss is a streaming reduction over N rows x 5 classes down to a
handful of per-class accumulators, followed by a trivial scalar combination.

Host side:
  - rows are partitioned by label t into 5 class groups (stable), split evenly
    across 8 cores, each per-core class segment padded to S = 128*R rows.
  - pivot transform: for a segment with label c the host sends the 4 columns
    y_i = x_i - x_c (i != c) as bf16, class-major blocks of width R. This
    both (a) drops DMA from 20B/row fp32 to 8B/row bf16 and (b) makes
    Z'+1 = Z / e^{x_c} = 1/p_c, so the per-row reciprocal needed by the conf
    term is free (no divide, no second exp).
  - per-core accumulators are summed on host (fp64) into the final scalar.

Device side (per core, SPMD over 8 cores), per class segment [128, 4R] bf16:
    E   = exp(Y)                        ScalarE (one LUT set: exp+ln)
    T1  = E01 + E23; Z' = T1a + T1b     DVE TT bf16 (2x mode, dense)
    dt  = ln(Z' + 1) (= -log p_c)       ScalarE (the +1 rides the Ln bias)
    c < 4:
      D4 = dt - z4  (= -log p_neg)      DVE TT
      U  = (Z'+1) * D4                  DVE ts(+1, 4x) + TT
      M  = (Z' < 1)  (= p_c > 0.5)      DVE ts (4x)
      G  = M * U                        DVE TT
    c == 4 (two half-tiles; dt = -log p_neg directly):
      T2/EM = max-tree of E             DVE TT
      G  = (2*EM - Z' <= 1)*(Z' >= 1)*dt   DVE ts/TT chain
    All sums go through TensorE: matmuls with per-class selector weights
    accumulate columns of G / M / z4 into PSUM [8, 4, 512] (num+li in
    regions 0-1 at po=c / po=4, den at po=c and sd at po=4+c in regions
    2-3). One DMA ships PSUM to DRAM; the host reduces in fp64.

Engines: ScalarE exp+ln only; DVE all 2x/4x-mode TT/ts ops (no 1x accum
ops); TensorE all accumulation; GpSimd unused (its SBUF-port contention
with DVE costs more than it saves).

Sentinel pads: +20 in all 4 cols for c<4 (masks off, z4=20 exactly
corrected on host), -20 for the negative segment (everything masks to 0).
"""

import numpy as np
import ml_dtypes

import concourse.bacc as bacc
import concourse.bass as bass
import concourse.mybir as mybir
import concourse.tile as tile
from concourse import bass_utils

F32 = mybir.dt.float32
BF16 = mybir.dt.bfloat16
Alu = mybir.AluOpType
Act = mybir.ActivationFunctionType

P = 128
NCLS = 5
N_CORES = 8
BANK = 512  # fp32 elements per PSUM bank per partition

BF = ml_dtypes.bfloat16

_PROGRAM_CACHE: dict[int, object] = {}


def _restrict_act_tables(arch: str):
    """Confine Exp/Ln to the natural_log_exp_and_others set so the act-table
    pass emits a single ACT_TABLE_LOAD instead of thrashing between sets."""
    from concourse import hw_specs

    tables = hw_specs.get_activation_tables(arch)
    if "natural_log_exp_and_others" not in tables:
        return
    for name, funcs in tables.items():
        if name != "natural_log_exp_and_others":
            funcs.discard(Act.Exp)
            funcs.discard(Act.Ln)


def _build_program(R: int):
    """Build + compile the per-core Bass program for segment width 4R."""
    assert R % 4 == 0
    H = R // 2  # half-tile block width for the negative segment
    HQ = H // 2  # matmul chunk width (must fit a PSUM bank: <= 512 fp32)
    assert H <= BANK
    nc = bacc.Bacc("TRN2", debug=False, num_devices=N_CORES)
    _restrict_act_tables(nc.m.arch)
    y_d = nc.dram_tensor("y", [NCLS, P, 4 * R], BF16, kind="ExternalInput").ap()
    w_d = nc.dram_tensor("w", [P, 64], BF16, kind="ExternalInput").ap()
    st_d = nc.dram_tensor("stats", [8, 4 * BANK], F32, kind="ExternalOutput").ap()

    # matmul bookkeeping: regions 0/1 = num (po=c) + li (po=4), halves lo/hi;
    # regions 2/3 = den (po=c) + sd (po=4+c), halves lo/hi.
    started = [False] * 4

    with tile.TileContext(nc) as tc:
        with (
            tc.tile_pool(name="io", bufs=6) as iop,
            tc.tile_pool(name="ep", bufs=3) as epp,
            tc.tile_pool(name="wk", bufs=3) as wp,
            tc.tile_pool(name="cn", bufs=1) as cp,
            tc.tile_pool(name="ps", bufs=1, space=bass.MemorySpace.PSUM) as pp,
        ):
            PS = pp.tile([8, 4, BANK], F32)
            Wt = cp.tile([P, 64], BF16)
            nc.sync.dma_start(out=Wt, in_=w_d)

            def mm(region, sel, rhs, last=False):
                """PS[:, region, :W] += Wt[:, 8*sel:8*sel+8].T @ rhs"""
                wid = rhs.shape[-1]
                nc.tensor.matmul(
                    PS[:, region, 0:wid],
                    Wt[:, 8 * sel : 8 * sel + 8],
                    rhs,
                    start=not started[region],
                    stop=last,
                )
                started[region] = True

            def neg_half(h, split_dma, last):
                """One half-tile of the negative segment (label = class 4)."""
                W = 2 * R * h  # column offset of this half in y_d[4]
                Y = iop.tile([P, 2 * R], BF16, tag=f"y4{h}", name="Y")
                E = epp.tile([P, 2 * R], BF16, tag="e4", name="E")
                if split_dma:
                    # two DMA+exp chunks so ScalarE starts earlier
                    for lo, hi in ((0, R), (R, 2 * R)):
                        nc.sync.dma_start(
                            out=Y[:, lo:hi], in_=y_d[4][:, W + lo : W + hi]
                        )
                        nc.scalar.activation(E[:, lo:hi], Y[:, lo:hi], Act.Exp)
                else:
                    nc.sync.dma_start(out=Y, in_=y_d[4][:, W : W + 2 * R])
                    nc.scalar.activation(E, Y, Act.Exp)
                T1 = wp.tile([P, R], BF16, tag="t1h", name="T1")
                nc.vector.tensor_tensor(
                    out=T1, in0=E[:, 0:R], in1=E[:, R : 2 * R], op=Alu.add
                )
                Z = wp.tile([P, H], BF16, tag="zh", name="Z")
                nc.vector.tensor_tensor(
                    out=Z, in0=T1[:, 0:H], in1=T1[:, H:R], op=Alu.add
                )
                # D4 = -log p_neg = ln(Z' + 1)
                D4 = wp.tile([P, H], BF16, tag="lnzh", name="D4")
                nc.scalar.activation(D4, Z, Act.Ln, bias=1.0)
                T2 = wp.tile([P, R], BF16, tag="t2", name="T2")
                nc.vector.tensor_tensor(
                    out=T2, in0=E[:, 0:R], in1=E[:, R : 2 * R], op=Alu.max
                )
                EM = wp.tile([P, H], BF16, tag="em", name="EM")
                nc.vector.tensor_tensor(
                    out=EM, in0=T2[:, 0:H], in1=T2[:, H:R], op=Alu.max
                )
                E2 = wp.tile([P, H], BF16, tag="e2", name="E2")
                nc.vector.tensor_scalar(
                    out=E2, in0=EM, scalar1=2.0, scalar2=None, op0=Alu.mult
                )
                # W4 = 2*EM - Z'; all positive p <= 0.5  <=>  W4 <= 1
                W4 = wp.tile([P, H], BF16, tag="w4", name="W4")
                nc.vector.tensor_tensor(out=W4, in0=E2, in1=Z, op=Alu.subtract)
                Ma = wp.tile([P, H], BF16, tag="ma", name="Ma")
                nc.vector.tensor_scalar(
                    out=Ma, in0=W4, scalar1=1.0, scalar2=None, op0=Alu.is_le
                )
                # Mb = (p_neg <= 0.5) * D4  <=>  (Z' >= 1) * D4
                Mb = wp.tile([P, H], BF16, tag="mb", name="Mb")
                nc.vector.tensor_scalar(
                    out=Mb, in0=Z, scalar1=1.0, scalar2=None, op0=Alu.is_ge
                )
                G1 = wp.tile([P, H], BF16, tag="g1", name="G1")
                nc.vector.tensor_tensor(out=G1, in0=Ma, in1=Mb, op=Alu.mult)
                G = wp.tile([P, H], BF16, tag="gh", name="G")
                nc.vector.tensor_tensor(out=G, in0=G1, in1=D4, op=Alu.mult)
                # li -> po=4 (selector 4) of num regions 0/1
                mm(0, 4, G[:, 0:HQ], last=last)
                mm(1, 4, G[:, HQ:H], last=last)

            def pos_seg(c, last):
                """One positive-class segment (label = class c < 4)."""
                Y = iop.tile([P, 4 * R], BF16, tag="y", name="Y")
                nc.sync.dma_start(out=Y, in_=y_d[c])
                E = epp.tile([P, 4 * R], BF16, tag="e", name="E")
                nc.scalar.activation(E, Y, Act.Exp)
                T1 = wp.tile([P, 2 * R], BF16, tag="t1", name="T1")
                nc.vector.tensor_tensor(
                    out=T1, in0=E[:, 0 : 2 * R], in1=E[:, 2 * R : 4 * R], op=Alu.add
                )
                Z = wp.tile([P, R], BF16, tag="z", name="Z")
                nc.vector.tensor_tensor(
                    out=Z, in0=T1[:, 0:R], in1=T1[:, R : 2 * R], op=Alu.add
                )
                # dt = -log p_c = ln(Z' + 1)
                DT = wp.tile([P, R], BF16, tag="lnz", name="DT")
                nc.scalar.activation(DT, Z, Act.Ln, bias=1.0)
                # D4 = -log p_neg = dt - z4
                D4 = wp.tile([P, R], BF16, tag="d4", name="D4")
                nc.vector.tensor_tensor(
                    out=D4, in0=DT, in1=Y[:, 3 * R : 4 * R], op=Alu.subtract
                )
                Z1 = wp.tile([P, R], BF16, tag="z1", name="Z1")
                nc.vector.tensor_scalar(
                    out=Z1, in0=Z, scalar1=1.0, scalar2=None, op0=Alu.add
                )
                U = wp.tile([P, R], BF16, tag="u", name="U")
                nc.vector.tensor_tensor(out=U, in0=Z1, in1=D4, op=Alu.mult)
                # p_c > 0.5  <=>  Z' < 1
                M = wp.tile([P, R], BF16, tag="mk", name="M")
                nc.vector.tensor_scalar(
                    out=M, in0=Z, scalar1=1.0, scalar2=None, op0=Alu.is_lt
                )
                G = wp.tile([P, R], BF16, tag="g", name="G")
                nc.vector.tensor_tensor(out=G, in0=M, in1=U, op=Alu.mult)
                # num -> po=c of regions 0/1 (regions 0/1 end at neg_half(1))
                mm(0, c, G[:, 0:H])
                mm(1, c, G[:, H:R])
                # den -> po=c of regions 2/3
                mm(2, c, M[:, 0:H])
                # sd -> po=4+c of regions 2/3 (sum of raw z4 columns)
                mm(2, 4 + c, Y[:, 3 * R : 3 * R + H], last=last)
                mm(3, c, M[:, H:R])
                mm(3, 4 + c, Y[:, 3 * R + H : 4 * R], last=last)

            # one short half first (primes the pipeline), one short half last
            STB = cp.tile([8, 4 * BANK], F32)
            neg_half(0, split_dma=True, last=False)
            for c in range(4):
                pos_seg(c, last=(c == 3))
            # regions 2/3 are complete after c=3: copy out under neg_half(1)
            nc.scalar.copy(
                STB[:, 2 * BANK : 4 * BANK],
                PS[:, 2:4, :].rearrange("p r b -> p (r b)"),
            )
            neg_half(1, split_dma=False, last=True)
            nc.scalar.copy(
                STB[:, 0 : 2 * BANK],
                PS[:, 0:2, :].rearrange("p r b -> p (r b)"),
            )
            nc.sync.dma_start(out=st_d, in_=STB)
    nc.compile()
    return nc


def _get_program(R: int):
    if R not in _PROGRAM_CACHE:
        _PROGRAM_CACHE[R] = _build_program(R)
    return _PROGRAM_CACHE[R]


def _make_selectors() -> np.ndarray:
    """[128, 64] bf16: block i (cols 8i..8i+8) = selector with column i set."""
    w = np.zeros((P, 64), dtype=BF)
    for i in range(8):
        w[:, 8 * i + i] = 1
    return w


def _prepare_inputs(x: np.ndarray, t: np.ndarray):
    """Sort rows by class, pivot-transform, shard across cores, pad segments.
    Returns (in_maps, counts, n_pad, R)."""
    N = x.shape[0]
    t64 = t.astype(np.int64, copy=False)
    counts = np.bincount(t64, minlength=NCLS).astype(np.int64)

    # per-core per-class row counts (even split of each class across cores)
    n_ck = np.zeros((NCLS, N_CORES), dtype=np.int64)
    for c in range(NCLS):
        q, r = divmod(int(counts[c]), N_CORES)
        n_ck[c] = q
        n_ck[c, :r] += 1

    R = int(max(8, -(-int(n_ck.max()) // P)))
    R = (R + 3) // 4 * 4  # multiple of 4: half tiles split into bank chunks
    S = P * R

    order = np.argsort(t64, kind="stable")
    xs = np.ascontiguousarray(x[order], dtype=np.float32)
    starts = np.concatenate([[0], np.cumsum(counts)])

    ycores = np.empty((N_CORES, NCLS, P, 4 * R), dtype=BF)
    for c in range(NCLS):
        off = int(starts[c])
        cols = [i for i in range(NCLS) if i != c]
        if c < 4:
            cols = [i for i in cols if i != 4] + [4]  # z4 in the last block
        pad = np.float32(20.0 if c < 4 else -20.0)
        for k in range(N_CORES):
            n = int(n_ck[c, k])
            ys = np.full((S, 4), pad, dtype=np.float32)
            if n:
                seg = xs[off : off + n]
                ys[:n] = seg[:, cols] - seg[:, c : c + 1]
                off += n
            if c < 4:
                # [S, 4] -> [128, 4R] class-major blocks of width R
                ycores[k, c] = (
                    ys.reshape(P, R, 4).transpose(0, 2, 1).reshape(P, 4 * R)
                )
            else:
                # two half-tiles, each [128, 4H] with H = R//2
                HH = R // 2
                for h in range(2):
                    half = ys[h * S // 2 : (h + 1) * S // 2]
                    ycores[k, c, :, h * 2 * R : (h + 1) * 2 * R] = (
                        half.reshape(P, HH, 4).transpose(0, 2, 1).reshape(P, 4 * HH)
                    )

    w = _make_selectors()
    in_maps = [{"y": ycores[k], "w": w} for k in range(N_CORES)]
    n_pad = N_CORES * S - counts  # per class, summed over cores
    return in_maps, counts, n_pad, R


def _combine(stats_list, counts, n_pad, N, R):
    """Host reduction of the PSUM accumulators + final scalar combination."""
    H = R // 2
    st = np.zeros((8, 4, BANK), dtype=np.float64)
    for s in stats_list:
        st += s.astype(np.float64).reshape(8, 4, BANK)
    # regions 0/1: num at po=c (summing lo+hi chunks), li at po=4
    acc = st[:, :, :H].sum(axis=2)  # [8 po, 4 regions]
    counts = counts.astype(np.float64)
    r13 = 0.0  # risk1 - risk3
    r2 = 0.0
    for c in range(4):
        num = acc[c, 0] + acc[c, 1]
        den = acc[c, 2] + acc[c, 3]
        sd = (acc[4 + c, 2] + acc[4 + c, 3]) - 20.0 * float(n_pad[c])
        prior = counts[c] / N
        r13 += prior * sd / max(1.0, counts[c])
        r2 += prior * num / max(den, 1.0)
    li = acc[4, 0] + acc[4, 1]
    r4 = li / max(1.0, counts[4])

    pos = 4.0 * (r13 + r2)
    if pos < 0.0:
        pos = 0.0
    return np.float32(pos + r4)


def run_device(in_maps, R, trace=False, **kw):
    nc = _get_program(R)
    res = bass_utils.run_bass_kernel_spmd(
        nc, in_maps, core_ids=list(range(N_CORES)), trace=trace, **kw
    )
    return res


def kernel(x: np.ndarray, t: np.ndarray) -> np.ndarray:
    x = np.asarray(x, dtype=np.float32)
    t = np.asarray(t)
    N = x.shape[0]
    in_maps, counts, n_pad, R = _prepare_inputs(x, t)
    res = run_device(in_maps, R)
    stats_list = [res.results[k]["stats"] for k in range(N_CORES)]
    return _combine(stats_list, counts, n_pad, N, R)
